# revision 38
# baseline (speedup 1.0000x reference)
"""Two-layer GCN (DGL norm='right') on 8 Trainium2 NeuronCores.

Strategy (graph/data parallel, per sharding hint):
  - Nodes are assigned to (core, block, slot) with degree-balanced blocks of 128.
  - Each core owns its nodes' incoming edges for both layers (dst-sharded).
  - Layer 1 gathers x[src] rows from a full replica of x via dma_gather; the
    segment-sum over edges is computed as one-hot matmuls accumulating
    aggT[feat, dst] in PSUM; then W1/relu/W2 run on-chip and the per-node
    z = relu(h) @ W2 rows (scaled by inv_deg) are written to a local shard.
  - An AllGather collective replicates the z shards to every core.
  - Layer 2 repeats the gather + one-hot matmul segment-sum over z rows and
    writes inv_deg-scaled output rows.
  - Normalization is algebraically postponed (relu(D^-1 M) = D^-1 relu(M))
    so all scaling is per-partition; b2 is added on host. Requires b1 == 0
    (guaranteed by the model inputs); a numpy fallback covers b1 != 0.
"""
import sys

sys.path.insert(0, "/opt/trn_rl_repo")

import numpy as np
from contextlib import ExitStack

import concourse.bass as bass
import concourse.tile as tile
from concourse import bacc, mybir
from concourse.masks import make_identity

# ----------------------------------------------------------------------------
# Configuration (hardcoded for the graded problem size)
# ----------------------------------------------------------------------------
CFG = dict(
    N=50000,       # nodes
    F=128,         # in/hidden features
    FOUT=64,       # output features
    NC=8,          # cores
    NBLK=49,       # dst blocks of 128 per core (49*128 = 6272 >= ceil(50000/8))
    P=128,
    GCHUNK=16,     # max tiles per dma_gather call (>8 uses single_packet=False)
    SB=4,          # blocks per superblock (gather-call batching unit)
    XSPLIT=25088,  # x table window split (< 32767 rows per window)
    NQ=1,          # SWDGE queues (1..4); gather calls round-robin across them
    L1BF16=True,   # gather x/build S in bf16 for layer 1
    ZBF16=True,    # z shards + allgather in bf16, cast to f32 on device
    AGCHUNKS=4,    # split the allgather into this many block-chunks
)


def _chunk_blocks(cfg):
    """Partition NBLK blocks into AGCHUNKS near-equal chunks.

    Returns (bounds, offs): bounds[c] = (b0, b1); offs[c] = row offset of
    chunk c's region in the gathered table (chunk-major, rank-minor).
    """
    NBLK, NC, P = cfg["NBLK"], cfg["NC"], cfg["P"]
    nch = cfg.get("AGCHUNKS", 1)
    base, rem = divmod(NBLK, nch)
    bounds, offs = [], []
    b0, off = 0, 0
    for c in range(nch):
        nb = base + (1 if c < rem else 0)
        bounds.append((b0, b0 + nb))
        offs.append(off)
        off += NC * nb * P
        b0 += nb
    return bounds, offs


# ----------------------------------------------------------------------------
# Host preprocessing
# ----------------------------------------------------------------------------
def _assign_nodes(deg, cfg):
    """Greedy degree-balanced assignment of nodes to (core, block, slot).

    Returns node_core, node_block, node_slot arrays and per-(core,block,slot)
    inverse mapping.
    """
    N, NC, NBLK, P = cfg["N"], cfg["NC"], cfg["NBLK"], cfg["P"]
    import heapq

    nbins = NC * NBLK
    order = np.argsort(-deg, kind="stable")
    bin_nodes = [[] for _ in range(nbins)]
    heap = [(0, i) for i in range(nbins)]
    heapq.heapify(heap)
    spill = []
    for n in order:
        d = int(deg[n])
        while True:
            load, i = heapq.heappop(heap)
            if len(bin_nodes[i]) < P:
                bin_nodes[i].append(n)
                if len(bin_nodes[i]) < P:
                    heapq.heappush(heap, (load + d, i))
                else:
                    spill.append((load + d, i))
                break
    # rank bins by load desc; i-th ranked bin -> core i%NC, block i//NC
    loads = np.zeros(nbins)
    for i in range(nbins):
        loads[i] = deg[bin_nodes[i]].sum() if bin_nodes[i] else 0
    rank = np.argsort(-loads, kind="stable")
    node_core = np.empty(N, np.int32)
    node_block = np.empty(N, np.int32)
    node_slot = np.empty(N, np.int32)
    for r, i in enumerate(rank):
        k, b = r % NC, r // NC
        nodes = bin_nodes[i]
        for s, n in enumerate(nodes):
            node_core[n] = k
            node_block[n] = b
            node_slot[n] = s
    return node_core, node_block, node_slot


def _build_layer_arrays(src_idx, win, ecore, eblock, eslotd, cfg):
    """Build per-core gather-index and dst-local arrays for one layer.

    Tiles are ordered superblock-major: for each superblock of SB blocks,
    first all window-0 groups, then all window-1 groups, so each window run
    can be fetched with a few large dma_gather calls.

    Returns dict with:
      idx16: [NC, 128, NSLOT//16] int16 (wrapped in 16 partitions, x8)
      dstl:  [NC, 128, TOTTILES] int32
      sbs:   per-superblock dict(ct0, SBT, calls=[(loc, col16, nt, w)],
             blocks=[(b, loc0, T0, loc1, T1)])
    """
    NC, NBLK, P = cfg["NC"], cfg["NBLK"], cfg["P"]
    GCH, SB = cfg["GCHUNK"], cfg["SB"]
    key = ((ecore.astype(np.int64) * NBLK + eblock) * 2 + win)
    ngroups = NC * NBLK * 2
    counts = np.bincount(key, minlength=ngroups).reshape(NC, NBLK * 2)
    need = (-(-counts.max(axis=0) // P)).astype(np.int64)  # [NBLK*2]
    # every block needs at least one tile so its PSUM gets written
    empty = (need[0::2] + need[1::2]) == 0
    need[0::2] = np.where(empty, 1, need[0::2])

    tile_base = np.zeros(NBLK * 2, np.int64)
    base = 0
    sbs = []
    for s0 in range(0, NBLK, SB):
        blocks = list(range(s0, min(s0 + SB, NBLK)))
        ct0 = base
        locs = {}
        calls = []
        for w in (0, 1):
            run_loc = base - ct0
            run_tiles = 0
            for b in blocks:
                bw = 2 * b + w
                tile_base[bw] = base
                locs[(b, w)] = base - ct0
                base += int(need[bw])
                run_tiles += int(need[bw])
            q = 0
            while q < run_tiles:
                nt = min(GCH, run_tiles - q)
                loc = run_loc + q
                calls.append((loc, (ct0 + loc) * (P // 16), nt, w))
                q += nt
        bmeta = [(b, locs[(b, 0)], int(need[2 * b]),
                  locs[(b, 1)], int(need[2 * b + 1])) for b in blocks]
        sbs.append(dict(ct0=ct0, SBT=base - ct0, calls=calls, blocks=bmeta))
    TOTTILES = base
    NSLOT = TOTTILES * P

    idx_arr = np.zeros((NC, NSLOT), np.int32)
    dstl_arr = np.full((NC, NSLOT), -1, np.int32)

    order = np.lexsort((np.arange(len(key)), key))
    skey = key[order]
    group_start_per_edge = np.searchsorted(skey, skey)
    pos = np.arange(len(skey)) - group_start_per_edge
    sk_core = skey // (NBLK * 2)
    sk_bw = skey % (NBLK * 2)
    slot = tile_base[sk_bw] * P + pos
    idx_arr[sk_core, slot] = src_idx[order]
    dstl_arr[sk_core, slot] = eslotd[order]

    i16 = idx_arr.astype(np.int16).reshape(NC, NSLOT // 16, 16)
    i16 = np.ascontiguousarray(i16.transpose(0, 2, 1))      # [NC, 16, NSLOT/16]
    idx16 = np.tile(i16, (1, 8, 1))                          # [NC, 128, NSLOT/16]
    dstl = np.ascontiguousarray(
        dstl_arr.reshape(NC, TOTTILES, P).transpose(0, 2, 1))

    return dict(idx16=idx16, dstl=dstl, sbs=sbs, TOTTILES=TOTTILES,
                SBTmax=max(s["SBT"] for s in sbs))


def _preprocess(x, src, dst, inv_deg, cfg):
    N, NC, NBLK, P = cfg["N"], cfg["NC"], cfg["NBLK"], cfg["P"]
    XS = cfg["XSPLIT"]
    node_core, node_block, node_slot = _assign_nodes(
        np.bincount(dst, minlength=N).astype(np.int64), cfg)

    # global padded id under chunk-major allgather layout
    bounds, offs = _chunk_blocks(cfg)
    chunk_of_block = np.empty(NBLK, np.int64)
    rows_c = np.empty(NBLK, np.int64)   # rows per core in the block's chunk
    b0_of_block = np.empty(NBLK, np.int64)
    off_of_block = np.empty(NBLK, np.int64)
    for c, (b0, b1) in enumerate(bounds):
        chunk_of_block[b0:b1] = c
        rows_c[b0:b1] = (b1 - b0) * P
        b0_of_block[b0:b1] = b0
        off_of_block[b0:b1] = offs[c]
    nb = node_block.astype(np.int64)
    gpid = (off_of_block[nb] + node_core.astype(np.int64) * rows_c[nb]
            + (nb - b0_of_block[nb]) * P + node_slot)

    ecore = node_core[dst]
    eblock = node_block[dst]
    eslotd = node_slot[dst]

    # layer 1: gather from x windows (original node ids)
    w1 = (src >= XS).astype(np.int64)
    sidx1 = (src - w1 * XS).astype(np.int32)
    L1 = _build_layer_arrays(sidx1, w1, ecore, eblock, eslotd, cfg)

    # layer 2: gather from z_full windows (global padded ids)
    ZS = (NC * NBLK * P) // 2  # 25088
    spid = gpid[src]
    w2 = (spid >= ZS).astype(np.int64)
    sidx2 = (spid - w2 * ZS).astype(np.int32)
    L2 = _build_layer_arrays(sidx2, w2, ecore, eblock, eslotd, cfg)

    # per-core inv_deg laid out [128, NBLK]
    invd = np.ones((NC, P, NBLK), np.float32)
    nodes = np.arange(N)
    invd[node_core, node_slot, node_block] = inv_deg[nodes]

    return dict(L1=L1, L2=L2, invd=invd,
                node_core=node_core, node_block=node_block,
                node_slot=node_slot)


# ----------------------------------------------------------------------------
# Bass program
# ----------------------------------------------------------------------------
def _build_program(pre, cfg, with_collective=True):
    N, F, FOUT, NC, NBLK, P = (cfg["N"], cfg["F"], cfg["FOUT"], cfg["NC"],
                               cfg["NBLK"], cfg["P"])
    XS = cfg["XSPLIT"]
    NLOC = NBLK * P
    NPAD = NC * NLOC
    ZS = NPAD // 2
    L1, L2 = pre["L1"], pre["L2"]
    f32, i32, i16 = mybir.dt.float32, mybir.dt.int32, mybir.dt.int16
    bf16 = mybir.dt.bfloat16
    l1bf = cfg.get("L1BF16", False)
    xdt = bf16 if l1bf else f32
    s1dt = bf16 if l1bf else f32
    d1dt = i16 if l1bf else i32
    NQ = cfg.get("NQ", 1)

    nc = bacc.Bacc("TRN2", target_bir_lowering=False, debug=False,
                   num_devices=NC if with_collective else 1,
                   num_swdge_queues=NQ)

    x_lo_d = nc.dram_tensor("x_lo", [XS, F], xdt, kind="ExternalInput").ap()
    x_hi_d = nc.dram_tensor("x_hi", [N - XS, F], xdt, kind="ExternalInput").ap()
    w1_d = nc.dram_tensor("w1", [F, F], f32, kind="ExternalInput").ap()
    w2_d = nc.dram_tensor("w2", [F, FOUT], f32, kind="ExternalInput").ap()
    invd_d = nc.dram_tensor("invd", [P, NBLK], f32, kind="ExternalInput").ap()
    i16a_d = nc.dram_tensor("i16a", list(L1["idx16"].shape[1:]), i16,
                            kind="ExternalInput").ap()
    dsa_d = nc.dram_tensor("dsa", list(L1["dstl"].shape[1:]), d1dt,
                           kind="ExternalInput").ap()
    i16b_d = nc.dram_tensor("i16b", list(L2["idx16"].shape[1:]), i16,
                            kind="ExternalInput").ap()
    dsb_d = nc.dram_tensor("dsb", list(L2["dstl"].shape[1:]), i32,
                           kind="ExternalInput").ap()
    out_d = nc.dram_tensor("out_local", [NLOC, FOUT], f32,
                           kind="ExternalOutput").ap()

    zbf = cfg.get("ZBF16", False)
    zdt = bf16 if zbf else f32
    bounds, offs = _chunk_blocks(cfg)
    nch = len(bounds)
    z_locals = [
        nc.dram_tensor(f"z_local{c}", [(b1 - b0) * P, FOUT], zdt).ap()
        for c, (b0, b1) in enumerate(bounds)
    ]
    if with_collective:
        z_gath = [
            nc.dram_tensor(f"z_gath{c}", [NC * (b1 - b0) * P, FOUT], zdt,
                           addr_space="Shared").ap()
            for c, (b0, b1) in enumerate(bounds)
        ]
        need_consolidate = zbf or nch > 1
        if need_consolidate:
            z_tab = nc.dram_tensor("z_tab", [NPAD, FOUT], f32).ap()
        else:
            z_tab = z_gath[0]
    else:
        z_tab = nc.dram_tensor("z_tab", [NPAD, FOUT], f32,
                               kind="ExternalInput").ap()
        need_consolidate = False
    chunk_of_block = {}
    for c, (b0, b1) in enumerate(bounds):
        for b in range(b0, b1):
            chunk_of_block[b] = c

    with tile.TileContext(nc) as tc, ExitStack() as ctx:
        const = ctx.enter_context(tc.tile_pool(name="const", bufs=1))

        iota_t = const.tile([P, P], dtype=i32)
        nc.gpsimd.iota(iota_t[:], pattern=[[1, P]], base=0, channel_multiplier=0)
        iota16_t = const.tile([P, P], dtype=i16)
        nc.gpsimd.iota(iota16_t[:], pattern=[[1, P]], base=0,
                       channel_multiplier=0)
        ident = const.tile([64, 64], dtype=f32)
        make_identity(nc, ident[:])
        w1_t = const.tile([F, F], dtype=f32)
        nc.sync.dma_start(out=w1_t[:], in_=w1_d[:])
        w2_t = const.tile([F, FOUT], dtype=f32)
        nc.sync.dma_start(out=w2_t[:], in_=w2_d[:])
        invd_t = const.tile([P, NBLK], dtype=f32)
        nc.sync.dma_start(out=invd_t[:], in_=invd_d[:])

        i16a_t = const.tile(list(L1["idx16"].shape[1:]), dtype=i16)
        nc.sync.dma_start(out=i16a_t[:], in_=i16a_d[:])
        dsa_t = const.tile(list(L1["dstl"].shape[1:]), dtype=d1dt)
        nc.sync.dma_start(out=dsa_t[:], in_=dsa_d[:])
        i16b_t = const.tile(list(L2["idx16"].shape[1:]), dtype=i16)
        nc.sync.dma_start(out=i16b_t[:], in_=i16b_d[:])
        dsb_t = const.tile(list(L2["dstl"].shape[1:]), dtype=i32)
        nc.sync.dma_start(out=dsb_t[:], in_=dsb_d[:])

        qrr = [0]

        def layer(lay, xwins, felem, dstl_t, i16_t, blk_tail,
                  gdt=f32, iot=iota_t, post_block=None):
            SBTmax = lay["SBTmax"]
            with ExitStack() as lctx:
                gp = lctx.enter_context(
                    tc.tile_pool(name="g", bufs=cfg.get("GBUFS", 2)))
                sp = lctx.enter_context(
                    tc.tile_pool(name="s", bufs=cfg.get("SBUFS", 2)))
                agp = lctx.enter_context(
                    tc.tile_pool(name="agg", bufs=cfg.get("AGGBUFS", 2),
                                 space="PSUM"))
                tp = lctx.enter_context(
                    tc.tile_pool(name="tail", bufs=2, space="PSUM"))
                tp2 = lctx.enter_context(
                    tc.tile_pool(name="tail2", bufs=2, space="PSUM"))
                sb = lctx.enter_context(tc.tile_pool(name="sb", bufs=2))

                skip = cfg.get("SKIP", ())
                for sblk in lay["sbs"]:
                    sbt, ct0 = sblk["SBT"], sblk["ct0"]
                    g_t = gp.tile([P, SBTmax, felem], dtype=gdt, tag="g")
                    if "gather" in skip:
                        nc.vector.memset(g_t[:], 0)
                    else:
                        for (loc, col16, nt, w) in sblk["calls"]:
                            nidx = nt * P
                            nc.gpsimd.dma_gather(
                                out_ap=g_t[:, loc:loc + nt, :],
                                in_ap=xwins[w],
                                idxs_ap=i16_t[:, col16:col16 + nidx // 16],
                                num_idxs=nidx,
                                num_idxs_reg=nidx,
                                elem_size=felem,
                                single_packet=(nidx <= 1024),
                                queue_num=qrr[0],
                            )
                            qrr[0] = (qrr[0] + 1) % NQ
                    s_t = sp.tile([P, SBTmax, P], dtype=gdt, tag="s")
                    if "sbuild" in skip:
                        nc.vector.memset(s_t[:], 0)
                    else:
                        nc.vector.tensor_tensor(
                            out=s_t[:, :sbt, :],
                            in0=dstl_t[:, ct0:ct0 + sbt, None].to_broadcast(
                                [P, sbt, P]),
                            in1=iot[:, None, :].to_broadcast([P, sbt, P]),
                            op=mybir.AluOpType.is_equal,
                        )
                    for (b, l0, t0, l1, t1) in sblk["blocks"]:
                        agg_ps = agp.tile([felem, P], dtype=f32, space="PSUM",
                                          tag="agg")
                        seq = list(range(l0, l0 + t0)) + \
                            list(range(l1, l1 + t1))
                        if "mm" in skip:
                            seq = seq[:1]
                        for i, t in enumerate(seq):
                            nc.tensor.matmul(
                                out=agg_ps[:],
                                lhsT=g_t[:, t, :],
                                rhs=s_t[:, t, :],
                                start=(i == 0),
                                stop=(i == len(seq) - 1),
                            )
                        if "tail" not in skip:
                            blk_tail(b, agg_ps, tp, tp2, sb)
                        if post_block is not None:
                            post_block(b)

        # ---------------- layer 1 ----------------
        def l1_tail(b, agg_ps, tp, tp2, sb):
            agg_sb = sb.tile([F, P], dtype=f32, tag="aggsb")
            nc.vector.tensor_copy(out=agg_sb[:], in_=agg_ps[:])
            h_ps = tp.tile([F, P], dtype=f32, space="PSUM", tag="h")
            nc.tensor.matmul(out=h_ps[:], lhsT=w1_t[:], rhs=agg_sb[:],
                             start=True, stop=True)
            r_sb = sb.tile([F, P], dtype=f32, tag="r")
            nc.scalar.activation(out=r_sb[:], in_=h_ps[:],
                                 func=mybir.ActivationFunctionType.Relu)
            z_ps = tp.tile([FOUT, P], dtype=f32, space="PSUM", tag="z")
            nc.tensor.matmul(out=z_ps[:], lhsT=w2_t[:], rhs=r_sb[:],
                             start=True, stop=True)
            z_sb = sb.tile([FOUT, P], dtype=f32, tag="zsb")
            nc.vector.tensor_copy(out=z_sb[:], in_=z_ps[:])
            zt_ps = tp2.tile([P, FOUT], dtype=f32, space="PSUM", tag="zt")
            nc.tensor.transpose(out=zt_ps[:], in_=z_sb[:], identity=ident[:])
            zrow = sb.tile([P, FOUT], dtype=zdt, tag="zrow")
            nc.scalar.activation(out=zrow[:], in_=zt_ps[:],
                                 func=mybir.ActivationFunctionType.Copy,
                                 scale=invd_t[:, b:b + 1])
            c = chunk_of_block[b]
            boff = (b - bounds[c][0]) * P
            nc.sync.dma_start(out=z_locals[c][boff:boff + P, :], in_=zrow[:])

        def l1_post(b):
            # after the last block of each chunk: allgather it (+ cast/copy
            # into the unified f32 gather table)
            if not with_collective:
                return
            for c, (b0, b1) in enumerate(bounds):
                if b == b1 - 1:
                    nc.gpsimd.collective_compute(
                        "AllGather",
                        mybir.AluOpType.bypass,
                        replica_groups=[list(range(NC))],
                        ins=[z_locals[c][:]],
                        outs=[z_gath[c][:]],
                    )
                    if need_consolidate:
                        nrows = z_gath[c].shape[0]
                        nc.gpsimd.dma_start(
                            out=z_tab[offs[c]:offs[c] + nrows, :],
                            in_=z_gath[c][:],
                        )

        layer(L1, (x_lo_d, x_hi_d), F, dsa_t, i16a_t, l1_tail,
              gdt=xdt, iot=(iota16_t if l1bf else iota_t),
              post_block=l1_post)

        # ---------------- layer 2 ----------------
        def l2_tail(b, agg_ps, tp, tp2, sb):
            o_sb = sb.tile([FOUT, P], dtype=f32, tag="osb")
            nc.vector.tensor_copy(out=o_sb[:], in_=agg_ps[:])
            ot_ps = tp2.tile([P, FOUT], dtype=f32, space="PSUM", tag="ot")
            nc.tensor.transpose(out=ot_ps[:], in_=o_sb[:], identity=ident[:])
            orow = sb.tile([P, FOUT], dtype=f32, tag="orow")
            nc.scalar.activation(out=orow[:], in_=ot_ps[:],
                                 func=mybir.ActivationFunctionType.Copy,
                                 scale=invd_t[:, b:b + 1])
            nc.sync.dma_start(out=out_d[b * P:(b + 1) * P, :], in_=orow[:])

        layer(L2, (z_tab[0:ZS, :], z_tab[ZS:NPAD, :]), FOUT, dsb_t, i16b_t,
              l2_tail)

    nc.compile()
    return nc


# ----------------------------------------------------------------------------
# Entry point
# ----------------------------------------------------------------------------
_CACHE = {}


def _numpy_fallback(x, src, dst, W1, b1, W2, b2):
    N = x.shape[0]
    deg = np.bincount(dst, minlength=N).astype(x.dtype)
    inv_deg = 1.0 / np.maximum(deg, 1.0)

    def gcn(xx, W, b):
        agg = np.zeros((N, xx.shape[1]), xx.dtype)
        np.add.at(agg, dst, xx[src])
        return agg * inv_deg[:, None] @ W + b

    h = np.maximum(gcn(x, W1, b1), 0.0)
    return gcn(h, W2, b2)


def kernel(x, src, dst, W1, b1, W2, b2):
    from concourse.bass_utils import run_bass_kernel_spmd

    cfg = CFG
    x = np.asarray(x, np.float32)
    src = np.asarray(src).astype(np.int64)
    dst = np.asarray(dst).astype(np.int64)
    W1 = np.asarray(W1, np.float32)
    b1 = np.asarray(b1, np.float32)
    W2 = np.asarray(W2, np.float32)
    b2 = np.asarray(b2, np.float32)

    if np.any(b1 != 0.0) or x.shape[0] != cfg["N"] or x.shape[1] != cfg["F"]:
        return _numpy_fallback(x, src, dst, W1, b1, W2, b2)

    N, NC, NBLK, P = cfg["N"], cfg["NC"], cfg["NBLK"], cfg["P"]
    deg = np.bincount(dst, minlength=N).astype(np.float32)
    inv_deg = (1.0 / np.maximum(deg, 1.0)).astype(np.float32)

    pre = _preprocess(x, src, dst, inv_deg, cfg)

    key = (pre["L1"]["TOTTILES"], pre["L2"]["TOTTILES"],
           tuple(s["SBT"] for s in pre["L1"]["sbs"]),
           tuple(s["SBT"] for s in pre["L2"]["sbs"]),
           tuple(sorted(cfg.items())))
    if key not in _CACHE:
        _CACHE[key] = _build_program(pre, cfg)
    nc = _CACHE[key]

    XS = cfg["XSPLIT"]
    if cfg.get("L1BF16", False):
        import ml_dtypes
        xc = x.astype(ml_dtypes.bfloat16)
        dsa_all = pre["L1"]["dstl"].astype(np.int16)
    else:
        xc = x
        dsa_all = pre["L1"]["dstl"]
    x_lo = np.ascontiguousarray(xc[:XS])
    x_hi = np.ascontiguousarray(xc[XS:])
    in_maps = []
    for k in range(NC):
        in_maps.append({
            "x_lo": x_lo, "x_hi": x_hi, "w1": W1, "w2": W2,
            "invd": pre["invd"][k],
            "i16a": pre["L1"]["idx16"][k], "dsa": dsa_all[k],
            "i16b": pre["L2"]["idx16"][k], "dsb": pre["L2"]["dstl"][k],
        })

    res = run_bass_kernel_spmd(nc, in_maps, core_ids=list(range(NC)))

    out = np.empty((N, cfg["FOUT"]), np.float32)
    rows = pre["node_block"].astype(np.int64) * P + pre["node_slot"]
    all_out = np.stack([res.results[k]["out_local"] for k in range(NC)])
    out[:] = all_out[pre["node_core"], rows]
    out += b2[None, :]
    return out


if __name__ == "__main__":
    # lightweight self-test of host preprocessing invariants
    rng = np.random.default_rng(0)
    N, E = CFG["N"], 800000
    src = rng.integers(0, N, E).astype(np.int64)
    dst = rng.integers(0, N, E).astype(np.int64)
    deg = np.bincount(dst, minlength=N).astype(np.float32)
    inv_deg = (1.0 / np.maximum(deg, 1.0)).astype(np.float32)
    pre = _preprocess(None, src, dst, inv_deg, CFG)
    for lname in ("L1", "L2"):
        lay = pre[lname]
        ncalls = sum(len(s["calls"]) for s in lay["sbs"])
        print(f"{lname}: TOTTILES {lay['TOTTILES']} SBTmax {lay['SBTmax']} "
              f"superblocks {len(lay['sbs'])} calls {ncalls}")
    print("shapes:", pre["L1"]["idx16"].shape, pre["L1"]["dstl"].shape)


# revision 39
# speedup vs baseline: 1.0176x; 1.0176x over previous
"""Two-layer GCN (DGL norm='right') on 8 Trainium2 NeuronCores.

Strategy (graph/data parallel, per sharding hint):
  - Nodes are assigned to (core, block, slot) with degree-balanced blocks of 128.
  - Each core owns its nodes' incoming edges for both layers (dst-sharded).
  - Layer 1 gathers x[src] rows from a full replica of x via dma_gather; the
    segment-sum over edges is computed as one-hot matmuls accumulating
    aggT[feat, dst] in PSUM; then W1/relu/W2 run on-chip and the per-node
    z = relu(h) @ W2 rows (scaled by inv_deg) are written to a local shard.
  - An AllGather collective replicates the z shards to every core.
  - Layer 2 repeats the gather + one-hot matmul segment-sum over z rows and
    writes inv_deg-scaled output rows.
  - Normalization is algebraically postponed (relu(D^-1 M) = D^-1 relu(M))
    so all scaling is per-partition; b2 is added on host. Requires b1 == 0
    (guaranteed by the model inputs); a numpy fallback covers b1 != 0.
"""
import sys

sys.path.insert(0, "/opt/trn_rl_repo")

import numpy as np
from contextlib import ExitStack

import concourse.bass as bass
import concourse.tile as tile
from concourse import bacc, mybir
from concourse.masks import make_identity

# ----------------------------------------------------------------------------
# Configuration (hardcoded for the graded problem size)
# ----------------------------------------------------------------------------
CFG = dict(
    N=50000,       # nodes
    F=128,         # in/hidden features
    FOUT=64,       # output features
    NC=8,          # cores
    NBLK=49,       # dst blocks of 128 per core (49*128 = 6272 >= ceil(50000/8))
    P=128,
    GCHUNK=8,      # max tiles per dma_gather call (1024 idxs, single_packet)
    SB=4,          # blocks per superblock (gather-call batching unit)
    XSPLIT=25088,  # x table window split (< 32767 rows per window)
    NQ=1,          # SWDGE queues (1..4); gather calls round-robin across them
    L1BF16=True,   # gather x/build S in bf16 for layer 1
    ZBF16=True,    # z shards + allgather in bf16, cast to f32 on device
    AGCHUNKS=4,    # split the allgather into this many block-chunks
)


def _chunk_blocks(cfg):
    """Partition NBLK blocks into AGCHUNKS near-equal chunks.

    Returns (bounds, offs): bounds[c] = (b0, b1); offs[c] = row offset of
    chunk c's region in the gathered table (chunk-major, rank-minor).
    """
    NBLK, NC, P = cfg["NBLK"], cfg["NC"], cfg["P"]
    nch = cfg.get("AGCHUNKS", 1)
    base, rem = divmod(NBLK, nch)
    bounds, offs = [], []
    b0, off = 0, 0
    for c in range(nch):
        nb = base + (1 if c < rem else 0)
        bounds.append((b0, b0 + nb))
        offs.append(off)
        off += NC * nb * P
        b0 += nb
    return bounds, offs


# ----------------------------------------------------------------------------
# Host preprocessing
# ----------------------------------------------------------------------------
def _assign_nodes(deg, cfg):
    """Greedy degree-balanced assignment of nodes to (core, block, slot).

    Returns node_core, node_block, node_slot arrays and per-(core,block,slot)
    inverse mapping.
    """
    N, NC, NBLK, P = cfg["N"], cfg["NC"], cfg["NBLK"], cfg["P"]
    import heapq

    nbins = NC * NBLK
    order = np.argsort(-deg, kind="stable")
    bin_nodes = [[] for _ in range(nbins)]
    heap = [(0, i) for i in range(nbins)]
    heapq.heapify(heap)
    spill = []
    for n in order:
        d = int(deg[n])
        while True:
            load, i = heapq.heappop(heap)
            if len(bin_nodes[i]) < P:
                bin_nodes[i].append(n)
                if len(bin_nodes[i]) < P:
                    heapq.heappush(heap, (load + d, i))
                else:
                    spill.append((load + d, i))
                break
    # rank bins by load desc; i-th ranked bin -> core i%NC, block i//NC
    loads = np.zeros(nbins)
    for i in range(nbins):
        loads[i] = deg[bin_nodes[i]].sum() if bin_nodes[i] else 0
    rank = np.argsort(-loads, kind="stable")
    node_core = np.empty(N, np.int32)
    node_block = np.empty(N, np.int32)
    node_slot = np.empty(N, np.int32)
    for r, i in enumerate(rank):
        k, b = r % NC, r // NC
        nodes = bin_nodes[i]
        for s, n in enumerate(nodes):
            node_core[n] = k
            node_block[n] = b
            node_slot[n] = s
    return node_core, node_block, node_slot


def _build_layer_arrays(src_idx, win, ecore, eblock, eslotd, cfg):
    """Build per-core gather-index and dst-local arrays for one layer.

    Tiles are ordered superblock-major: for each superblock of SB blocks,
    first all window-0 groups, then all window-1 groups, so each window run
    can be fetched with a few large dma_gather calls.

    Returns dict with:
      idx16: [NC, 128, NSLOT//16] int16 (wrapped in 16 partitions, x8)
      dstl:  [NC, 128, TOTTILES] int32
      sbs:   per-superblock dict(ct0, SBT, calls=[(loc, col16, nt, w)],
             blocks=[(b, loc0, T0, loc1, T1)])
    """
    NC, NBLK, P = cfg["NC"], cfg["NBLK"], cfg["P"]
    GCH, SB = cfg["GCHUNK"], cfg["SB"]
    key = ((ecore.astype(np.int64) * NBLK + eblock) * 2 + win)
    ngroups = NC * NBLK * 2
    counts = np.bincount(key, minlength=ngroups).reshape(NC, NBLK * 2)
    need = (-(-counts.max(axis=0) // P)).astype(np.int64)  # [NBLK*2]
    # every block needs at least one tile so its PSUM gets written
    empty = (need[0::2] + need[1::2]) == 0
    need[0::2] = np.where(empty, 1, need[0::2])

    tile_base = np.zeros(NBLK * 2, np.int64)
    base = 0
    sbs = []
    for s0 in range(0, NBLK, SB):
        blocks = list(range(s0, min(s0 + SB, NBLK)))
        ct0 = base
        locs = {}
        calls = []
        for w in (0, 1):
            run_loc = base - ct0
            run_tiles = 0
            for b in blocks:
                bw = 2 * b + w
                tile_base[bw] = base
                locs[(b, w)] = base - ct0
                base += int(need[bw])
                run_tiles += int(need[bw])
            q = 0
            while q < run_tiles:
                nt = min(GCH, run_tiles - q)
                loc = run_loc + q
                calls.append((loc, (ct0 + loc) * (P // 16), nt, w))
                q += nt
        bmeta = [(b, locs[(b, 0)], int(need[2 * b]),
                  locs[(b, 1)], int(need[2 * b + 1])) for b in blocks]
        sbs.append(dict(ct0=ct0, SBT=base - ct0, calls=calls, blocks=bmeta))
    TOTTILES = base
    NSLOT = TOTTILES * P

    idx_arr = np.zeros((NC, NSLOT), np.int32)
    dstl_arr = np.full((NC, NSLOT), -1, np.int32)

    order = np.lexsort((np.arange(len(key)), key))
    skey = key[order]
    group_start_per_edge = np.searchsorted(skey, skey)
    pos = np.arange(len(skey)) - group_start_per_edge
    sk_core = skey // (NBLK * 2)
    sk_bw = skey % (NBLK * 2)
    slot = tile_base[sk_bw] * P + pos
    idx_arr[sk_core, slot] = src_idx[order]
    dstl_arr[sk_core, slot] = eslotd[order]

    i16 = idx_arr.astype(np.int16).reshape(NC, NSLOT // 16, 16)
    i16 = np.ascontiguousarray(i16.transpose(0, 2, 1))      # [NC, 16, NSLOT/16]
    idx16 = np.tile(i16, (1, 8, 1))                          # [NC, 128, NSLOT/16]
    dstl = np.ascontiguousarray(
        dstl_arr.reshape(NC, TOTTILES, P).transpose(0, 2, 1))

    return dict(idx16=idx16, dstl=dstl, sbs=sbs, TOTTILES=TOTTILES,
                SBTmax=max(s["SBT"] for s in sbs))


def _preprocess(x, src, dst, inv_deg, cfg):
    N, NC, NBLK, P = cfg["N"], cfg["NC"], cfg["NBLK"], cfg["P"]
    XS = cfg["XSPLIT"]
    node_core, node_block, node_slot = _assign_nodes(
        np.bincount(dst, minlength=N).astype(np.int64), cfg)

    # global padded id under chunk-major allgather layout
    bounds, offs = _chunk_blocks(cfg)
    chunk_of_block = np.empty(NBLK, np.int64)
    rows_c = np.empty(NBLK, np.int64)   # rows per core in the block's chunk
    b0_of_block = np.empty(NBLK, np.int64)
    off_of_block = np.empty(NBLK, np.int64)
    for c, (b0, b1) in enumerate(bounds):
        chunk_of_block[b0:b1] = c
        rows_c[b0:b1] = (b1 - b0) * P
        b0_of_block[b0:b1] = b0
        off_of_block[b0:b1] = offs[c]
    nb = node_block.astype(np.int64)
    gpid = (off_of_block[nb] + node_core.astype(np.int64) * rows_c[nb]
            + (nb - b0_of_block[nb]) * P + node_slot)

    ecore = node_core[dst]
    eblock = node_block[dst]
    eslotd = node_slot[dst]

    # layer 1: gather from x windows (original node ids)
    w1 = (src >= XS).astype(np.int64)
    sidx1 = (src - w1 * XS).astype(np.int32)
    L1 = _build_layer_arrays(sidx1, w1, ecore, eblock, eslotd, cfg)

    # layer 2: gather from z_full windows (global padded ids)
    ZS = (NC * NBLK * P) // 2  # 25088
    spid = gpid[src]
    w2 = (spid >= ZS).astype(np.int64)
    sidx2 = (spid - w2 * ZS).astype(np.int32)
    L2 = _build_layer_arrays(sidx2, w2, ecore, eblock, eslotd, cfg)

    # per-core inv_deg laid out [128, NBLK]
    invd = np.ones((NC, P, NBLK), np.float32)
    nodes = np.arange(N)
    invd[node_core, node_slot, node_block] = inv_deg[nodes]

    return dict(L1=L1, L2=L2, invd=invd,
                node_core=node_core, node_block=node_block,
                node_slot=node_slot)


# ----------------------------------------------------------------------------
# Bass program
# ----------------------------------------------------------------------------
def _build_program(pre, cfg, with_collective=True):
    N, F, FOUT, NC, NBLK, P = (cfg["N"], cfg["F"], cfg["FOUT"], cfg["NC"],
                               cfg["NBLK"], cfg["P"])
    XS = cfg["XSPLIT"]
    NLOC = NBLK * P
    NPAD = NC * NLOC
    ZS = NPAD // 2
    L1, L2 = pre["L1"], pre["L2"]
    f32, i32, i16 = mybir.dt.float32, mybir.dt.int32, mybir.dt.int16
    bf16 = mybir.dt.bfloat16
    l1bf = cfg.get("L1BF16", False)
    xdt = bf16 if l1bf else f32
    s1dt = bf16 if l1bf else f32
    d1dt = i16 if l1bf else i32
    NQ = cfg.get("NQ", 1)

    nc = bacc.Bacc("TRN2", target_bir_lowering=False, debug=False,
                   num_devices=NC if with_collective else 1,
                   num_swdge_queues=NQ)

    x_lo_d = nc.dram_tensor("x_lo", [XS, F], xdt, kind="ExternalInput").ap()
    x_hi_d = nc.dram_tensor("x_hi", [N - XS, F], xdt, kind="ExternalInput").ap()
    w1_d = nc.dram_tensor("w1", [F, F], f32, kind="ExternalInput").ap()
    w2_d = nc.dram_tensor("w2", [F, FOUT], f32, kind="ExternalInput").ap()
    invd_d = nc.dram_tensor("invd", [P, NBLK], f32, kind="ExternalInput").ap()
    i16a_d = nc.dram_tensor("i16a", list(L1["idx16"].shape[1:]), i16,
                            kind="ExternalInput").ap()
    dsa_d = nc.dram_tensor("dsa", list(L1["dstl"].shape[1:]), d1dt,
                           kind="ExternalInput").ap()
    i16b_d = nc.dram_tensor("i16b", list(L2["idx16"].shape[1:]), i16,
                            kind="ExternalInput").ap()
    dsb_d = nc.dram_tensor("dsb", list(L2["dstl"].shape[1:]), i32,
                           kind="ExternalInput").ap()
    out_d = nc.dram_tensor("out_local", [NLOC, FOUT], f32,
                           kind="ExternalOutput").ap()

    zbf = cfg.get("ZBF16", False)
    zdt = bf16 if zbf else f32
    bounds, offs = _chunk_blocks(cfg)
    nch = len(bounds)
    z_locals = [
        nc.dram_tensor(f"z_local{c}", [(b1 - b0) * P, FOUT], zdt).ap()
        for c, (b0, b1) in enumerate(bounds)
    ]
    if with_collective:
        z_gath = [
            nc.dram_tensor(f"z_gath{c}", [NC * (b1 - b0) * P, FOUT], zdt,
                           addr_space="Shared").ap()
            for c, (b0, b1) in enumerate(bounds)
        ]
        need_consolidate = zbf or nch > 1
        if need_consolidate:
            z_tab = nc.dram_tensor("z_tab", [NPAD, FOUT], f32).ap()
        else:
            z_tab = z_gath[0]
    else:
        z_tab = nc.dram_tensor("z_tab", [NPAD, FOUT], f32,
                               kind="ExternalInput").ap()
        need_consolidate = False
    chunk_of_block = {}
    for c, (b0, b1) in enumerate(bounds):
        for b in range(b0, b1):
            chunk_of_block[b] = c

    with tile.TileContext(nc) as tc, ExitStack() as ctx:
        const = ctx.enter_context(tc.tile_pool(name="const", bufs=1))

        iota_t = const.tile([P, P], dtype=i32)
        nc.gpsimd.iota(iota_t[:], pattern=[[1, P]], base=0, channel_multiplier=0)
        iota16_t = const.tile([P, P], dtype=i16)
        nc.gpsimd.iota(iota16_t[:], pattern=[[1, P]], base=0,
                       channel_multiplier=0)
        ident = const.tile([64, 64], dtype=f32)
        make_identity(nc, ident[:])
        w1_t = const.tile([F, F], dtype=f32)
        nc.sync.dma_start(out=w1_t[:], in_=w1_d[:])
        w2_t = const.tile([F, FOUT], dtype=f32)
        nc.sync.dma_start(out=w2_t[:], in_=w2_d[:])
        invd_t = const.tile([P, NBLK], dtype=f32)
        nc.sync.dma_start(out=invd_t[:], in_=invd_d[:])

        i16a_t = const.tile(list(L1["idx16"].shape[1:]), dtype=i16)
        nc.sync.dma_start(out=i16a_t[:], in_=i16a_d[:])
        dsa_t = const.tile(list(L1["dstl"].shape[1:]), dtype=d1dt)
        nc.sync.dma_start(out=dsa_t[:], in_=dsa_d[:])
        i16b_t = const.tile(list(L2["idx16"].shape[1:]), dtype=i16)
        nc.sync.dma_start(out=i16b_t[:], in_=i16b_d[:])
        dsb_t = const.tile(list(L2["dstl"].shape[1:]), dtype=i32)
        nc.sync.dma_start(out=dsb_t[:], in_=dsb_d[:])

        qrr = [0]

        def layer(lay, xwins, felem, dstl_t, i16_t, blk_tail,
                  gdt=f32, iot=iota_t, post_block=None):
            SBTmax = lay["SBTmax"]
            with ExitStack() as lctx:
                gp = lctx.enter_context(
                    tc.tile_pool(name="g", bufs=cfg.get("GBUFS", 2)))
                sp = lctx.enter_context(
                    tc.tile_pool(name="s", bufs=cfg.get("SBUFS", 2)))
                agp = lctx.enter_context(
                    tc.tile_pool(name="agg", bufs=cfg.get("AGGBUFS", 2),
                                 space="PSUM"))
                tp = lctx.enter_context(
                    tc.tile_pool(name="tail", bufs=2, space="PSUM"))
                tp2 = lctx.enter_context(
                    tc.tile_pool(name="tail2", bufs=2, space="PSUM"))
                sb = lctx.enter_context(tc.tile_pool(name="sb", bufs=2))

                skip = cfg.get("SKIP", ())
                for sblk in lay["sbs"]:
                    sbt, ct0 = sblk["SBT"], sblk["ct0"]
                    g_t = gp.tile([P, SBTmax, felem], dtype=gdt, tag="g")
                    if "gather" in skip:
                        nc.vector.memset(g_t[:], 0)
                    else:
                        for (loc, col16, nt, w) in sblk["calls"]:
                            nidx = nt * P
                            nc.gpsimd.dma_gather(
                                out_ap=g_t[:, loc:loc + nt, :],
                                in_ap=xwins[w],
                                idxs_ap=i16_t[:, col16:col16 + nidx // 16],
                                num_idxs=nidx,
                                num_idxs_reg=nidx,
                                elem_size=felem,
                                single_packet=(nidx <= 1024),
                                queue_num=qrr[0],
                            )
                            qrr[0] = (qrr[0] + 1) % NQ
                    s_t = sp.tile([P, SBTmax, P], dtype=gdt, tag="s")
                    if "sbuild" in skip:
                        nc.vector.memset(s_t[:], 0)
                    else:
                        nc.vector.tensor_tensor(
                            out=s_t[:, :sbt, :],
                            in0=dstl_t[:, ct0:ct0 + sbt, None].to_broadcast(
                                [P, sbt, P]),
                            in1=iot[:, None, :].to_broadcast([P, sbt, P]),
                            op=mybir.AluOpType.is_equal,
                        )
                    for (b, l0, t0, l1, t1) in sblk["blocks"]:
                        agg_ps = agp.tile([felem, P], dtype=f32, space="PSUM",
                                          tag="agg")
                        seq = list(range(l0, l0 + t0)) + \
                            list(range(l1, l1 + t1))
                        if "mm" in skip:
                            seq = seq[:1]
                        for i, t in enumerate(seq):
                            nc.tensor.matmul(
                                out=agg_ps[:],
                                lhsT=g_t[:, t, :],
                                rhs=s_t[:, t, :],
                                start=(i == 0),
                                stop=(i == len(seq) - 1),
                            )
                        if "tail" not in skip:
                            blk_tail(b, agg_ps, tp, tp2, sb)
                        if post_block is not None:
                            post_block(b)

        # ---------------- layer 1 ----------------
        def l1_tail(b, agg_ps, tp, tp2, sb):
            agg_sb = sb.tile([F, P], dtype=f32, tag="aggsb")
            nc.vector.tensor_copy(out=agg_sb[:], in_=agg_ps[:])
            h_ps = tp.tile([F, P], dtype=f32, space="PSUM", tag="h")
            nc.tensor.matmul(out=h_ps[:], lhsT=w1_t[:], rhs=agg_sb[:],
                             start=True, stop=True)
            r_sb = sb.tile([F, P], dtype=f32, tag="r")
            nc.scalar.activation(out=r_sb[:], in_=h_ps[:],
                                 func=mybir.ActivationFunctionType.Relu)
            z_ps = tp.tile([FOUT, P], dtype=f32, space="PSUM", tag="z")
            nc.tensor.matmul(out=z_ps[:], lhsT=w2_t[:], rhs=r_sb[:],
                             start=True, stop=True)
            z_sb = sb.tile([FOUT, P], dtype=f32, tag="zsb")
            nc.vector.tensor_copy(out=z_sb[:], in_=z_ps[:])
            zt_ps = tp2.tile([P, FOUT], dtype=f32, space="PSUM", tag="zt")
            nc.tensor.transpose(out=zt_ps[:], in_=z_sb[:], identity=ident[:])
            zrow = sb.tile([P, FOUT], dtype=zdt, tag="zrow")
            nc.scalar.activation(out=zrow[:], in_=zt_ps[:],
                                 func=mybir.ActivationFunctionType.Copy,
                                 scale=invd_t[:, b:b + 1])
            c = chunk_of_block[b]
            boff = (b - bounds[c][0]) * P
            nc.sync.dma_start(out=z_locals[c][boff:boff + P, :], in_=zrow[:])

        def l1_post(b):
            # after the last block of each chunk: allgather it (+ cast/copy
            # into the unified f32 gather table)
            if not with_collective:
                return
            for c, (b0, b1) in enumerate(bounds):
                if b == b1 - 1:
                    nc.gpsimd.collective_compute(
                        "AllGather",
                        mybir.AluOpType.bypass,
                        replica_groups=[list(range(NC))],
                        ins=[z_locals[c][:]],
                        outs=[z_gath[c][:]],
                    )
                    if need_consolidate:
                        nrows = z_gath[c].shape[0]
                        nc.gpsimd.dma_start(
                            out=z_tab[offs[c]:offs[c] + nrows, :],
                            in_=z_gath[c][:],
                        )

        layer(L1, (x_lo_d, x_hi_d), F, dsa_t, i16a_t, l1_tail,
              gdt=xdt, iot=(iota16_t if l1bf else iota_t),
              post_block=l1_post)

        # ---------------- layer 2 ----------------
        def l2_tail(b, agg_ps, tp, tp2, sb):
            o_sb = sb.tile([FOUT, P], dtype=f32, tag="osb")
            nc.vector.tensor_copy(out=o_sb[:], in_=agg_ps[:])
            ot_ps = tp2.tile([P, FOUT], dtype=f32, space="PSUM", tag="ot")
            nc.tensor.transpose(out=ot_ps[:], in_=o_sb[:], identity=ident[:])
            orow = sb.tile([P, FOUT], dtype=f32, tag="orow")
            nc.scalar.activation(out=orow[:], in_=ot_ps[:],
                                 func=mybir.ActivationFunctionType.Copy,
                                 scale=invd_t[:, b:b + 1])
            nc.sync.dma_start(out=out_d[b * P:(b + 1) * P, :], in_=orow[:])

        layer(L2, (z_tab[0:ZS, :], z_tab[ZS:NPAD, :]), FOUT, dsb_t, i16b_t,
              l2_tail)

    nc.compile()
    return nc


# ----------------------------------------------------------------------------
# Entry point
# ----------------------------------------------------------------------------
_CACHE = {}


def _numpy_fallback(x, src, dst, W1, b1, W2, b2):
    N = x.shape[0]
    deg = np.bincount(dst, minlength=N).astype(x.dtype)
    inv_deg = 1.0 / np.maximum(deg, 1.0)

    def gcn(xx, W, b):
        agg = np.zeros((N, xx.shape[1]), xx.dtype)
        np.add.at(agg, dst, xx[src])
        return agg * inv_deg[:, None] @ W + b

    h = np.maximum(gcn(x, W1, b1), 0.0)
    return gcn(h, W2, b2)


def kernel(x, src, dst, W1, b1, W2, b2):
    from concourse.bass_utils import run_bass_kernel_spmd

    cfg = CFG
    x = np.asarray(x, np.float32)
    src = np.asarray(src).astype(np.int64)
    dst = np.asarray(dst).astype(np.int64)
    W1 = np.asarray(W1, np.float32)
    b1 = np.asarray(b1, np.float32)
    W2 = np.asarray(W2, np.float32)
    b2 = np.asarray(b2, np.float32)

    if np.any(b1 != 0.0) or x.shape[0] != cfg["N"] or x.shape[1] != cfg["F"]:
        return _numpy_fallback(x, src, dst, W1, b1, W2, b2)

    N, NC, NBLK, P = cfg["N"], cfg["NC"], cfg["NBLK"], cfg["P"]
    deg = np.bincount(dst, minlength=N).astype(np.float32)
    inv_deg = (1.0 / np.maximum(deg, 1.0)).astype(np.float32)

    pre = _preprocess(x, src, dst, inv_deg, cfg)

    key = (pre["L1"]["TOTTILES"], pre["L2"]["TOTTILES"],
           tuple(s["SBT"] for s in pre["L1"]["sbs"]),
           tuple(s["SBT"] for s in pre["L2"]["sbs"]),
           tuple(sorted(cfg.items())))
    if key not in _CACHE:
        _CACHE[key] = _build_program(pre, cfg)
    nc = _CACHE[key]

    XS = cfg["XSPLIT"]
    if cfg.get("L1BF16", False):
        import ml_dtypes
        xc = x.astype(ml_dtypes.bfloat16)
        dsa_all = pre["L1"]["dstl"].astype(np.int16)
    else:
        xc = x
        dsa_all = pre["L1"]["dstl"]
    x_lo = np.ascontiguousarray(xc[:XS])
    x_hi = np.ascontiguousarray(xc[XS:])
    in_maps = []
    for k in range(NC):
        in_maps.append({
            "x_lo": x_lo, "x_hi": x_hi, "w1": W1, "w2": W2,
            "invd": pre["invd"][k],
            "i16a": pre["L1"]["idx16"][k], "dsa": dsa_all[k],
            "i16b": pre["L2"]["idx16"][k], "dsb": pre["L2"]["dstl"][k],
        })

    res = run_bass_kernel_spmd(nc, in_maps, core_ids=list(range(NC)))

    out = np.empty((N, cfg["FOUT"]), np.float32)
    rows = pre["node_block"].astype(np.int64) * P + pre["node_slot"]
    all_out = np.stack([res.results[k]["out_local"] for k in range(NC)])
    out[:] = all_out[pre["node_core"], rows]
    out += b2[None, :]
    return out


if __name__ == "__main__":
    # lightweight self-test of host preprocessing invariants
    rng = np.random.default_rng(0)
    N, E = CFG["N"], 800000
    src = rng.integers(0, N, E).astype(np.int64)
    dst = rng.integers(0, N, E).astype(np.int64)
    deg = np.bincount(dst, minlength=N).astype(np.float32)
    inv_deg = (1.0 / np.maximum(deg, 1.0)).astype(np.float32)
    pre = _preprocess(None, src, dst, inv_deg, CFG)
    for lname in ("L1", "L2"):
        lay = pre[lname]
        ncalls = sum(len(s["calls"]) for s in lay["sbs"])
        print(f"{lname}: TOTTILES {lay['TOTTILES']} SBTmax {lay['SBTmax']} "
              f"superblocks {len(lay['sbs'])} calls {ncalls}")
    print("shapes:", pre["L1"]["idx16"].shape, pre["L1"]["dstl"].shape)
